# revision 29
# baseline (speedup 1.0000x reference)
"""Multi-head attention kernel for 8 TRN2 NeuronCores (raw Bass, no Tile).

Problem: x[2,4096,256] -> qkv proj -> 8-head attention (Dh=32) -> out proj.
Sharding: 16 (batch, head) pairs over 8 cores: core i handles batch i//4 and
heads {2*(i%4), 2*(i%4)+1}. Each core computes its 2 heads' attention plus the
partial output projection; host sums the 4 partial projections per batch.

Per-core design:
  - qkv projections in bf16 (x and weights host-cast; all weights arrive in
    one packed DMA; bias via ones-row augmented matmuls).  q is pre-scaled by
    SCALE*log2(e) host-side so scores come out as z = log2(e)*s.
  - v projected first (8 bank-aligned PSUM slots, copies split ACT/DVE, one
    strided 3-D copy per k-block), then q/k (4 slots, k-copies on DVE,
    q-copies on ACT as activation-Copy with float32r output).
  - scores in float32r (the copies provide the BIR-required f32r rounding):
    S^T[128 kpos, 512 q] tiles at 1 cycle/row.
  - exp split across two engines (the throughput bottleneck):
      ACT: exact exp via activation(Exp, scale=ln2) -> bf16
      DVE: Schraudolph bit-trick: int16(z*128 + 16251.25) bitcast to bf16
           (exact round-to-nearest on HW; softmax normalization cancels most
           of the +-4% sawtooth)
  - AV in bf16 with a packed v [v_h(32)|ones(32)] per (k-block, head) so the
    matmul both accumulates AV and broadcasts the softmax denominator:
    po[64, 512] = rows 0:32 AV, rows 32:64 denominator (replicated).
  - epilogue: DVE reciprocal(den) + ACT copy(AV) + DVE SBUF mul -> oT bf16.
  - out proj in bf16 from oT; partials DMA'd as f32 [256, 4096].

Pipeline: 3 score PSUM slots + AV lagging scores by 2 tiles keep the PE from
round-tripping on exp latency; 4 exp SBUF slots decouple the exp engines; the
two out-proj halves straddle an iteration boundary so the PSUM->SBUF copy
between them hides under attention tiles.
PSUM banks: s0,s1,s2 (2 each) + po_both (1; halves are the po ping-pong) +
pq0 (1).  The prologue reuses all of them as projection slots.
"""

import sys

sys.path.insert(0, "/opt/trn_rl_repo")

import numpy as np

B, N, C, H, Dh = 2, 4096, 256, 8, 32
HPC = 2  # heads per core
NCORES = 8
SCALE = C ** -0.5
LOG2E = float(np.log2(np.e))
LN2 = float(np.log(2.0))
SIGMA = -4.75  # Schraudolph mean-centering shift
QC = 512  # q columns per chunk
NQC = N // QC  # 8
KB = 128  # k rows per block
NKB = N // KB  # 32
NI = NQC * HPC  # 16 (c, h) iterations
NT = NKB // 2  # 16 double-kb tiles per iteration
NG = NI * NT  # 256 score/exp/AV tiles
WPK = 1472  # packed weight tensor columns
IDR = 2  # first iteration using fp8 DoubleRow for odd-t score tiles
XC = 1024  # x DMA chunk columns
NXC = N // XC  # 4 chunk pairs

# exp tile assignment: per-iteration t values handled by DVE (Schraudolph);
# the rest go to ACT (exact exp).
DVE_T = (2, 5, 7, 9, 11, 13, 15)

_CACHE = {}


def _build_nc():
    import concourse.bass as bass
    import concourse.mybir as mybir

    f32 = mybir.dt.float32
    f32r = mybir.dt.float32r
    bf16 = mybir.dt.bfloat16
    i16 = mybir.dt.int16
    EXP = mybir.ActivationFunctionType.Exp
    Alu = mybir.AluOpType

    # exp engine assignment tables
    asg = [1 if (g % NT) in DVE_T else 0 for g in range(NG)]  # 1 = DVE
    # tail chain: last exp on ACT (the po copy follows there)
    asg[NG - 1] = 0
    preA = [0] * (NG + 1)
    preD = [0] * (NG + 1)
    for g in range(NG):
        preA[g + 1] = preA[g] + (asg[g] == 0)
        preD[g + 1] = preD[g] + (asg[g] == 1)

    nc = bass.Bass("TRN2", target_bir_lowering=False, debug=False, num_devices=NCORES)

    ins = {}
    for nm, shp, dt_ in (
        ("wpack", [128, WPK], bf16),
        ("x0", [128, N], bf16),
        ("x1", [128, N], bf16),
    ):
        ins[nm] = nc.dram_tensor(nm, shp, dt_, kind="ExternalInput")
    out = nc.dram_tensor("out", [C, N], f32, kind="ExternalOutput")

    from contextlib import ExitStack
    with ExitStack() as ctx:
        E = ctx.enter_context
        xt0 = E(nc.sbuf_tensor("xt0", [128, N], bf16))
        xt1 = E(nc.sbuf_tensor("xt1", [128, N], bf16))
        wpk = E(nc.sbuf_tensor("wpk", [128, WPK], bf16))
        ones_sm = E(nc.sbuf_tensor("ones_sm", [1, QC], bf16))
        qT = E(nc.sbuf_tensor("qT", [128, N], f32r))
        kT = E(nc.sbuf_tensor("kT", [128, N], f32r))
        f8 = mybir.dt.float8e4
        qd8 = E(nc.sbuf_tensor("qd8", [64, 2, N], f8))
        kd8 = E(nc.sbuf_tensor("kd8", [64, 2, N], f8))
        v_all = E(nc.sbuf_tensor("v_all", [128, NKB, 2, 2 * Dh], bf16))
        pt0 = E(nc.sbuf_tensor("pt0", [128, 2 * QC], bf16))
        pt1 = E(nc.sbuf_tensor("pt1", [128, 2 * QC], bf16))
        pt2 = E(nc.sbuf_tensor("pt2", [128, 2 * QC], bf16))
        pt3 = E(nc.sbuf_tensor("pt3", [128, 2 * QC], bf16))
        oT0 = E(nc.sbuf_tensor("oT0", [Dh, N], bf16))
        oT1 = E(nc.sbuf_tensor("oT1", [Dh, N], bf16))
        po_sb0 = E(nc.sbuf_tensor("po_sb0", [Dh, QC], f32))
        po_sb1 = E(nc.sbuf_tensor("po_sb1", [Dh, QC], f32))
        rsb = E(nc.sbuf_tensor("rsb", [Dh, QC], f32))
        osb0 = E(nc.sbuf_tensor("osb0", [128, QC], f32))
        osb1 = E(nc.sbuf_tensor("osb1", [128, QC], f32))
        s0 = E(nc.psum_tensor("s0", [128, 2 * QC], f32))
        s1 = E(nc.psum_tensor("s1", [128, 2 * QC], f32))
        s2 = E(nc.psum_tensor("s2", [128, 2 * QC], f32))
        po_both = E(nc.psum_tensor("po_both", [128, QC], f32))
        pq0 = E(nc.psum_tensor("pq0", [128, QC], f32))
        dIN = E(nc.semaphore("dIN"))
        mset = E(nc.semaphore("mset"))
        sPQ = E(nc.semaphore("sPQ"))
        sCP = E(nc.semaphore("sCP"))  # DVE production copies
        sCA = E(nc.semaphore("sCA"))  # ACT production copies
        sSC = E(nc.semaphore("sSC"))
        sEXA = E(nc.semaphore("sEXA"))
        sEXD = E(nc.semaphore("sEXD"))
        sAV = E(nc.semaphore("sAV"))
        sPOC = E(nc.semaphore("sPOC"))
        sRC = E(nc.semaphore("sRC"))
        sMU = E(nc.semaphore("sMU"))
        sPJ = E(nc.semaphore("sPJ"))
        sOS = E(nc.semaphore("sOS"))
        sOD0 = E(nc.semaphore("sOD0"))
        sOD1 = E(nc.semaphore("sOD1"))
        sF8K = E(nc.semaphore("sF8K"))
        sF8Q = E(nc.semaphore("sF8Q"))
        block = E(nc.Block())

        s_ = (s0, s1, s2)
        pt_ = (pt0, pt1, pt2, pt3)
        pt_ap = tuple(t.ap() for t in pt_)
        po_sb_ = (po_sb0, po_sb1)
        osb_ = (osb0, osb1)
        oT_ = (oT0, oT1)

        w = wpk.ap()
        twq0, twk0, twv0 = w[:, 0:128], w[:, 128:256], w[:, 256:320]
        twq1, twk1, twv1 = w[:, 320:448], w[:, 448:576], w[:, 576:640]
        twqb, twkb, twvb = w[0:1, 640:768], w[0:1, 768:896], w[0:1, 896:960]
        twp0, twp1 = w[0:Dh, 960:1216], w[0:Dh, 1216:1472]

        pob = po_both.ap()

        def po_av(i):  # AV rows of the po ping-pong half
            return pob[0:Dh, :] if i % 2 == 0 else pob[64 : 64 + Dh, :]

        def po_full(i):
            return pob[0 : 2 * Dh, :] if i % 2 == 0 else pob[64:128, :]

        def po_den(i):
            return pob[Dh : 2 * Dh, :] if i % 2 == 0 else pob[64 + Dh : 128, :]

        # prologue projection slots (all bank starts)
        sa = (s0.ap(), s1.ap(), s2.ap())
        v_slots = (
            pq0.ap(), pob, sa[0], sa[0][:, QC:], sa[1], sa[1][:, QC:],
            sa[2], sa[2][:, QC:],
        )
        qk_slots = (pq0.ap(), pob, sa[0], sa[1])

        def kc_of(t):  # k chunk needed by score tile t
            return (2 * t + 1) // 4

        def wait_exp(eng, g):
            """Wait until exp(g) is done (engine-specific counter)."""
            if g < 0:
                return
            if asg[g] == 0:
                eng.wait_ge(sEXA, preA[g] + 1)
            else:
                eng.wait_ge(sEXD, preD[g] + 1)

        @block.sync
        def _(sync):
            sync.dma_start(out=wpk[:, :], in_=ins["wpack"].ap()).then_inc(dIN, 16)
            for j in range(NXC):  # x chunks, interleaved halves
                sl = slice(j * XC, (j + 1) * XC)
                sync.dma_start(out=xt0[:, sl], in_=ins["x0"].ap()[:, sl]).then_inc(
                    dIN, 16
                )
                sync.dma_start(out=xt1[:, sl], in_=ins["x1"].ap()[:, sl]).then_inc(
                    dIN, 16
                )
            for k in range(2 * NQC):  # out DMAs: c-major, mc-minor
                c, mc = k // 2, k % 2
                sync.wait_ge(sOS, k + 1)
                sync.dma_start(
                    out=out.ap()[mc * 128 : (mc + 1) * 128, c * QC : (c + 1) * QC],
                    in_=osb_[k % 2][:, :],
                ).then_inc((sOD0, sOD1)[k % 2], 16)

        @block.gpsimd
        def _(gpsimd):
            gpsimd.memset(ones_sm[:, :], 1.0).then_inc(mset, 1)
            # only the ones-halves; the v-halves are written by the copies
            gpsimd.memset(v_all[:, :, :, Dh : 2 * Dh], 1.0).then_inc(mset, 1)
            # fp8 pair-packed copies from the f32r tensors (k first: DR tiles
            # of iteration IDR need all k chunks early).  The source rows
            # include the zero-padding rows, so no fp8 memsets are needed.
            for c in range(NQC):
                sl = slice(c * QC, (c + 1) * QC)
                gpsimd.wait_ge(sCP, 24 + c + 1)  # kT chunk c copied
                for h in (0, 1):
                    for j_ in (0, 1):
                        gpsimd.tensor_copy(
                            kd8.ap()[32 * h : 32 * h + 32, j_, sl],
                            kT.ap()[64 * h + 32 * j_ : 64 * h + 32 * j_ + 32, sl]
                            .bitcast(f32),
                        ).then_inc(sF8K, 1)
            for c in range(NQC):
                sl = slice(c * QC, (c + 1) * QC)
                gpsimd.wait_ge(sCA, 8 + c + 1)  # qT chunk c copied
                for h in (0, 1):
                    for j_ in (0, 1):
                        gpsimd.tensor_copy(
                            qd8.ap()[32 * h : 32 * h + 32, j_, sl],
                            qT.ap()[64 * h + 32 * j_ : 64 * h + 32 * j_ + 32, sl]
                            .bitcast(f32),
                        ).then_inc(sF8Q, 1)

        @block.tensor
        def _(tensor):
            tensor.wait_ge(mset, 1)

            # ---- v projection first: 32 blocks over 8 slots, chasing DMA ----
            for pb in range(NKB):
                psl = slice(pb * KB, (pb + 1) * KB)
                tensor.wait_ge(dIN, 16 + 32 * (pb // 8 + 1))
                if pb >= 8:  # slot pb-8's copy done
                    pr_ = pb - 8
                    if pr_ % 4 == 3:
                        tensor.wait_ge(sCA, pr_ // 4 + 1)
                    else:
                        tensor.wait_ge(sCP, pr_ + 1 - (pr_ + 1) // 4)
                pv = v_slots[pb % 8][0:128, 0:64]
                tensor.matmul(pv, xt0[:, psl], twv0, start=True, stop=False)
                tensor.matmul(pv, xt1[:, psl], twv1, start=False, stop=False)
                tensor.matmul(
                    pv, ones_sm[0:1, 0:KB], twvb, start=False, stop=True
                ).then_inc(sPQ, 1)

            # ---- q/k projections: 16 groups, c-major, k then q ----
            for g in range(2 * NQC):
                c, which = g // 2, g % 2  # which: 0 = k, 1 = q
                sl = slice(c * QC, (c + 1) * QC)
                w0, w1, wb = ((twk0, twk1, twkb), (twq0, twq1, twqb))[which]
                if g < 4:  # slot still held by the v phase
                    tensor.wait_ge(sCP, 24)
                    tensor.wait_ge(sCA, 8)
                else:
                    base = (24, 8)[(g - 4) % 2]
                    tensor.wait_ge((sCP, sCA)[(g - 4) % 2], base + (g - 4) // 2 + 1)
                p = qk_slots[g % 4][0:128, 0:QC]
                tensor.matmul(p, w0, xt0[:, sl], start=True, stop=False)
                tensor.matmul(p, w1, xt1[:, sl], start=False, stop=False)
                tensor.matmul(
                    p, wb, ones_sm[:, :], start=False, stop=True
                ).then_inc(sPQ, 1)

            def scores(g):
                i, t = g // NT, g % NT
                c, h = i // 2, i % 2
                hsl = slice(64 * h, 64 * h + 64)
                h32 = slice(32 * h, 32 * h + 32)
                qsl = slice(c * QC, (c + 1) * QC)
                use_dr = i >= IDR and t % 2 == 1
                if i == 0:
                    tensor.wait_ge(sCP, 25 + max(kc_of(t), 1 if t == 0 else 0))
                    tensor.wait_ge(sCA, 10 if t == 1 else 9)
                if use_dr:
                    # fp8 copies for the needed k chunk + q chunk ready
                    tensor.wait_ge(sF8K, 4 * (kc_of(t) + 1))
                    tensor.wait_ge(sF8Q, 4 * (c + 1))
                wait_exp(tensor, g - 3)  # s[g%3] free after exp g-3
                for j in (0, 1):
                    kb = 2 * t + j
                    ksl = slice(kb * KB, (kb + 1) * KB)
                    if use_dr:
                        mm = tensor.matmul(
                            s_[g % 3][:, j * QC : (j + 1) * QC],
                            kd8.ap()[h32, :, ksl],
                            qd8.ap()[h32, :, qsl],
                            start=True,
                            stop=True,
                            perf_mode=mybir.MatmulPerfMode.DoubleRow,
                        )
                    else:
                        mm = tensor.matmul(
                            s_[g % 3][:, j * QC : (j + 1) * QC],
                            kT[hsl, ksl],
                            qT[hsl, qsl],
                            start=True,
                            stop=True,
                        )
                    if j == 1:
                        mm.then_inc(sSC, 1)

            def av(g):
                i, t = g // NT, g % NT
                h = i % 2
                wait_exp(tensor, g)
                if g == 0:  # po/pq banks still held by qk-copy readers
                    tensor.wait_ge(sCP, 32)
                    tensor.wait_ge(sCA, 16)
                    tensor.wait_ge(mset, 2)  # v_all ones columns ready
                if t == 0 and i >= 2:
                    tensor.wait_ge(sPOC, i - 1)  # po half free (ACT copy done)
                    tensor.wait_ge(sRC, i - 1)  # and recip done
                for j in (0, 1):
                    kb = 2 * t + j
                    mm = tensor.matmul(
                        po_full(i),
                        v_all[:, kb, h, :],
                        pt_ap[g % 4][:, j * QC : (j + 1) * QC],
                        start=(kb == 0),
                        stop=(kb == NKB - 1),
                        skip_group_check=True,
                    )
                    if j == 1:
                        mm.then_inc(sAV, 1)

            def proj_mc(c, mc):
                qsl = slice(c * QC, (c + 1) * QC)
                if mc == 0:
                    tensor.wait_ge(sMU, 2 * c + 2)
                k = 2 * c + mc
                msl = slice(mc * 128, (mc + 1) * 128)
                if k == 2 * NQC - 1:
                    dst = pob[0:128, 0:QC]  # po retired; skip the osb wait
                else:
                    if k >= 1:
                        tensor.wait_ge(sOS, k)  # pq0 free after osb copy k-1
                    dst = pq0[:, :]
                tensor.matmul(
                    dst, twp0[:, msl], oT0[:, qsl], start=True, stop=False
                )
                tensor.matmul(
                    dst, twp1[:, msl], oT1[:, qsl], start=False, stop=True
                ).then_inc(sPJ, 1)

            for g in range(NG):
                scores(g)
                if g >= 2:
                    av(g - 2)
                    j = g - 2
                    if j % NT == NT - 1:
                        i_done = j // NT
                        if i_done >= 2 and i_done % 2 == 0:
                            proj_mc((i_done - 2) // 2, 0)
                    if j % NT == 1:
                        i = j // NT
                        if i >= 3 and i % 2 == 1:
                            proj_mc((i - 3) // 2, 1)
            av(NG - 2)
            av(NG - 1)
            proj_mc(NQC - 1, 0)
            proj_mc(NQC - 1, 1)

        @block.scalar
        def _(scalar):
            # ACT: every 4th v copy + q copies (f32r), then exp + po copies.
            for pb in range(3, NKB, 4):
                scalar.wait_ge(sPQ, pb + 1)
                pv = v_slots[pb % 8][0:128, 0:64]
                scalar.copy(
                    v_all.ap()[:, pb, :, 0:Dh], pv[:, 0 : 2 * Dh]
                ).then_inc(sCA, 1)
            def q_copy(c):
                g = 2 * c + 1
                sl = slice(c * QC, (c + 1) * QC)
                scalar.wait_ge(sPQ, NKB + g + 1)
                scalar.copy(
                    qT[:, sl], qk_slots[g % 4][0:128, 0:QC]
                ).then_inc(sCA, 1)

            for c_ in range(NQC):
                q_copy(c_)

            def po_copy(i):
                scalar.wait_ge(sAV, NT * (i + 1))
                if i >= 2:
                    scalar.wait_ge(sMU, i - 1)  # po_sb[i%2] reader done
                scalar.copy(po_sb_[i % 2][:, :], po_av(i)).then_inc(sPOC, 1)

            def osb_copy_act(k):
                scalar.wait_ge(sPJ, k + 1)
                src_ = pob[0:128, 0:QC] if k == 2 * NQC - 1 else pq0[:, :]
                scalar.copy(osb_[k % 2][:, :], src_).then_inc(sOS, 1)

            pending = -1  # iteration whose po_copy is pending
            for g in range(NG):
                i, t = g // NT, g % NT
                if asg[g] == 0:
                    na = preA[g + 1] - preA[i * NT]  # ACT-tile index within iter
                    scalar.wait_ge(sSC, g + 1)
                    if g >= 4:
                        scalar.wait_ge(sAV, g - 3)  # pt[g%4] free after av g-4
                    scalar.activation(
                        pt_[g % 4][:, :], s_[g % 3][:, :], EXP, 0.0, LN2
                    ).then_inc(sEXA, 1)
                    if i >= 1 and na == 4 and pending == i - 1:
                        po_copy(i - 1)
                        pending = -2
                if t == NT - 1:
                    if pending >= 0:
                        po_copy(pending)
                    pending = i
            po_copy(NI - 1)
            osb_copy_act(2 * NQC - 1)

        @block.vector
        def _(vector):
            # three of every four v copies, then k copies (f32r)
            for pb in range(NKB):
                if pb % 4 == 3:
                    continue
                vector.wait_ge(sPQ, pb + 1)
                pv = v_slots[pb % 8][0:128, 0:64]
                vector.tensor_copy(
                    v_all.ap()[:, pb, :, 0:Dh], pv[:, 0 : 2 * Dh]
                ).then_inc(sCP, 1)
            def k_copy(c):
                g = 2 * c
                sl = slice(c * QC, (c + 1) * QC)
                vector.wait_ge(sPQ, NKB + g + 1)
                vector.tensor_copy(
                    kT[:, sl], qk_slots[g % 4][0:128, 0:QC]
                ).then_inc(sCP, 1)

            for c_ in range(NQC):
                k_copy(c_)

            def exp_dve(g):
                vector.wait_ge(sSC, g + 1)
                if g >= 4:
                    vector.wait_ge(sAV, g - 3)
                vector.tensor_scalar(
                    pt_ap[g % 4][:, :].bitcast(i16),
                    s_[g % 3][:, :],
                    128.0,
                    16256.0 + SIGMA,
                    Alu.mult,
                    Alu.add,
                ).then_inc(sEXD, 1)

            def epi_recip(i):
                vector.wait_ge(sAV, NT * (i + 1))
                vector.reciprocal(rsb[:, :], po_den(i)).then_inc(sRC, 1)

            def epi_mul(i):
                h = i % 2
                c = i // 2
                qsl = slice(c * QC, (c + 1) * QC)
                vector.wait_ge(sPOC, i + 1)
                vector.tensor_mul(
                    oT_[h][:, qsl], po_sb_[i % 2][:, :], rsb[:, :]
                ).then_inc(sMU, 1)

            def epilogue(i):
                epi_recip(i)
                epi_mul(i)

            def osb_copy(k):
                vector.wait_ge(sPJ, k + 1)
                if k >= 2:
                    vector.wait_ge((sOD0, sOD1)[k % 2], 16 * (k // 2))
                src_ = pob[0:128, 0:QC] if k == 2 * NQC - 1 else pq0[:, :]
                vector.tensor_copy(osb_[k % 2][:, :], src_).then_inc(sOS, 1)

            for i in range(NI):
                for t in range(NT):
                    g = i * NT + t
                    if asg[g] == 1:
                        exp_dve(g)
                        nd = preD[g + 1] - preD[i * NT]
                        if i >= 1 and nd == 1:
                            epi_recip(i - 1)
                        if i >= 1 and nd == 2:
                            epi_mul(i - 1)
                        if i >= 3 and i % 2 == 1 and nd == 3:
                            osb_copy(i - 2)
                # osb copy for proj mc=0 lands right after this iteration's
                # last DVE exp so the PE's next sOS wait can resolve
                if i >= 2 and i % 2 == 0:
                    osb_copy(i - 2)
            epilogue(NI - 1)
            osb_copy(2 * NQC - 2)

    return nc


def _prep_in_maps(x, W_qkv, b_qkv, W_proj):
    import ml_dtypes

    bf = ml_dtypes.bfloat16
    in_maps = []
    for i in range(NCORES):
        b = i // 4
        heads = [2 * (i % 4), 2 * (i % 4) + 1]
        xT = np.ascontiguousarray(x[b].T.astype(np.float32))  # [256, 4096]

        def slc(base, scale):
            w = np.concatenate(
                [W_qkv[:, base + h * Dh : base + (h + 1) * Dh] for h in heads], axis=1
            ).astype(np.float32) * scale
            bb = np.concatenate(
                [b_qkv[base + h * Dh : base + (h + 1) * Dh] for h in heads]
            ).astype(np.float32)[None, :] * scale
            return w, bb

        wq, bq = slc(0, SCALE * LOG2E)
        wk, bk = slc(C, 1.0)
        wv, bv = slc(2 * C, 1.0)
        wp = np.concatenate(
            [W_proj[h * Dh : (h + 1) * Dh, :] for h in heads], axis=0
        ).astype(np.float32)  # [64, 256]

        def pad_qk(w64):
            # [rows, 64] -> [rows, 128]: head h dim d=2p+j -> col 64h+32j+p,
            # cols 64h+32j+16 .. +32 stay zero (DoubleRow pair padding)
            o = np.zeros((w64.shape[0], 128), dtype=np.float32)
            for h in (0, 1):
                wh = w64[:, 32 * h : 32 * h + 32]
                for j in (0, 1):
                    o[:, 64 * h + 32 * j : 64 * h + 32 * j + 16] = wh[:, j::2]
            return o

        wqp, wkp = pad_qk(wq), pad_qk(wk)
        bqp, bkp = pad_qk(bq), pad_qk(bk)

        wpack = np.zeros((128, WPK), dtype=np.float32)
        wpack[:, 0:128] = wqp[:128]
        wpack[:, 128:256] = wkp[:128]
        wpack[:, 256:320] = wv[:128]
        wpack[:, 320:448] = wqp[128:]
        wpack[:, 448:576] = wkp[128:]
        wpack[:, 576:640] = wv[128:]
        wpack[0, 640:768] = bqp[0]
        wpack[0, 768:896] = bkp[0]
        wpack[0, 896:960] = bv[0]
        wpack[0:Dh, 960:1216] = wp[:Dh]
        wpack[0:Dh, 1216:1472] = wp[Dh:]

        m = {
            "wpack": wpack.astype(bf),
            "x0": xT[:128].astype(bf),
            "x1": xT[128:].astype(bf),
        }
        in_maps.append({k: np.ascontiguousarray(v) for k, v in m.items()})
    return in_maps


LAST_RESULT = None


def kernel(x, W_qkv, b_qkv, W_proj, b_proj):
    global LAST_RESULT
    from concourse.bass_utils import run_bass_kernel_spmd

    if "nc" not in _CACHE:
        _CACHE["nc"] = _build_nc()
    nc = _CACHE["nc"]

    in_maps = _prep_in_maps(
        np.asarray(x), np.asarray(W_qkv), np.asarray(b_qkv), np.asarray(W_proj)
    )
    res = run_bass_kernel_spmd(nc, in_maps, core_ids=list(range(NCORES)))
    LAST_RESULT = res
    outs = res.results
    full = np.zeros((B, N, C), dtype=np.float32)
    for i in range(NCORES):
        b = i // 4
        full[b] += np.asarray(outs[i]["out"]).T
    full += np.asarray(b_proj).astype(np.float32)[None, None, :]
    return full


# revision 30
# speedup vs baseline: 1.0274x; 1.0274x over previous
"""Multi-head attention kernel for 8 TRN2 NeuronCores (raw Bass, no Tile).

Problem: x[2,4096,256] -> qkv proj -> 8-head attention (Dh=32) -> out proj.
Sharding: 16 (batch, head) pairs over 8 cores: core i handles batch i//4 and
heads {2*(i%4), 2*(i%4)+1}. Each core computes its 2 heads' attention plus the
partial output projection; host sums the 4 partial projections per batch.

Per-core design:
  - qkv projections in bf16 (x and weights host-cast; all weights arrive in
    one packed DMA; bias via ones-row augmented matmuls).  q is pre-scaled by
    SCALE*log2(e) host-side so scores come out as z = log2(e)*s.
  - v projected first (8 bank-aligned PSUM slots, copies split ACT/DVE, one
    strided 3-D copy per k-block), then q/k (4 slots, k-copies on DVE,
    q-copies on ACT as activation-Copy with float32r output).
  - scores in float32r (the copies provide the BIR-required f32r rounding):
    S^T[128 kpos, 512 q] tiles at 1 cycle/row.
  - exp split across two engines (the throughput bottleneck):
      ACT: exact exp via activation(Exp, scale=ln2) -> bf16
      DVE: Schraudolph bit-trick: int16(z*128 + 16251.25) bitcast to bf16
           (exact round-to-nearest on HW; softmax normalization cancels most
           of the +-4% sawtooth)
  - AV in bf16 with a packed v [v_h(32)|ones(32)] per (k-block, head) so the
    matmul both accumulates AV and broadcasts the softmax denominator:
    po[64, 512] = rows 0:32 AV, rows 32:64 denominator (replicated).
  - epilogue: DVE reciprocal(den) + ACT copy(AV) + DVE SBUF mul -> oT bf16.
  - out proj in bf16 from oT; partials DMA'd as f32 [256, 4096].

Pipeline: 3 score PSUM slots + AV lagging scores by 2 tiles keep the PE from
round-tripping on exp latency; 4 exp SBUF slots decouple the exp engines; the
two out-proj halves straddle an iteration boundary so the PSUM->SBUF copy
between them hides under attention tiles.
PSUM banks: s0,s1,s2 (2 each) + po_both (1; halves are the po ping-pong) +
pq0 (1).  The prologue reuses all of them as projection slots.
"""

import sys

sys.path.insert(0, "/opt/trn_rl_repo")

import numpy as np

B, N, C, H, Dh = 2, 4096, 256, 8, 32
HPC = 2  # heads per core
NCORES = 8
SCALE = C ** -0.5
LOG2E = float(np.log2(np.e))
LN2 = float(np.log(2.0))
SIGMA = -4.75  # Schraudolph mean-centering shift
QC = 512  # q columns per chunk
NQC = N // QC  # 8
KB = 128  # k rows per block
NKB = N // KB  # 32
NI = NQC * HPC  # 16 (c, h) iterations
NT = NKB // 2  # 16 double-kb tiles per iteration
NG = NI * NT  # 256 score/exp/AV tiles
WPK = 1472  # packed weight tensor columns
IDR = 2  # first iteration using fp8 DoubleRow for odd-t score tiles
XC = 1024  # x DMA chunk columns
NXC = N // XC  # 4 chunk pairs

# exp tile assignment: per-iteration t values handled by DVE (Schraudolph);
# the rest go to ACT (exact exp).
DVE_T = (2, 5, 7, 9, 11, 13, 15)

_CACHE = {}


def _build_nc():
    import concourse.bass as bass
    import concourse.mybir as mybir

    f32 = mybir.dt.float32
    f32r = mybir.dt.float32r
    bf16 = mybir.dt.bfloat16
    i16 = mybir.dt.int16
    EXP = mybir.ActivationFunctionType.Exp
    Alu = mybir.AluOpType

    # exp engine assignment tables
    asg = [1 if (g % NT) in DVE_T else 0 for g in range(NG)]  # 1 = DVE
    # tail chain: last exp on ACT (the po copy follows there)
    asg[NG - 1] = 0
    preA = [0] * (NG + 1)
    preD = [0] * (NG + 1)
    for g in range(NG):
        preA[g + 1] = preA[g] + (asg[g] == 0)
        preD[g + 1] = preD[g] + (asg[g] == 1)

    nc = bass.Bass("TRN2", target_bir_lowering=False, debug=False, num_devices=NCORES)

    ins = {}
    for nm, shp, dt_ in (
        ("wpack", [128, WPK], bf16),
        ("x0", [128, N], bf16),
        ("x1", [128, N], bf16),
    ):
        ins[nm] = nc.dram_tensor(nm, shp, dt_, kind="ExternalInput")
    out = nc.dram_tensor("out", [C, N], f32, kind="ExternalOutput")

    from contextlib import ExitStack
    with ExitStack() as ctx:
        E = ctx.enter_context
        xt0 = E(nc.sbuf_tensor("xt0", [128, N], bf16))
        xt1 = E(nc.sbuf_tensor("xt1", [128, N], bf16))
        wpk = E(nc.sbuf_tensor("wpk", [128, WPK], bf16))
        ones_sm = E(nc.sbuf_tensor("ones_sm", [1, QC], bf16))
        qT = E(nc.sbuf_tensor("qT", [128, N], f32r))
        kT = E(nc.sbuf_tensor("kT", [128, N], f32r))
        f8 = mybir.dt.float8e4
        qd8 = E(nc.sbuf_tensor("qd8", [64, 2, N], f8))
        kd8 = E(nc.sbuf_tensor("kd8", [64, 2, N], f8))
        v_all = E(nc.sbuf_tensor("v_all", [128, NKB, 2, 2 * Dh], bf16))
        pt0 = E(nc.sbuf_tensor("pt0", [128, 2 * QC], bf16))
        pt1 = E(nc.sbuf_tensor("pt1", [128, 2 * QC], bf16))
        pt2 = E(nc.sbuf_tensor("pt2", [128, 2 * QC], bf16))
        pt3 = E(nc.sbuf_tensor("pt3", [128, 2 * QC], bf16))
        oT0 = E(nc.sbuf_tensor("oT0", [Dh, N], bf16))
        oT1 = E(nc.sbuf_tensor("oT1", [Dh, N], bf16))
        po_sb0 = E(nc.sbuf_tensor("po_sb0", [Dh, QC], f32))
        po_sb1 = E(nc.sbuf_tensor("po_sb1", [Dh, QC], f32))
        rsb = E(nc.sbuf_tensor("rsb", [Dh, QC], f32))
        osb0 = E(nc.sbuf_tensor("osb0", [128, QC], f32))
        osb1 = E(nc.sbuf_tensor("osb1", [128, QC], f32))
        s0 = E(nc.psum_tensor("s0", [128, 2 * QC], f32))
        s1 = E(nc.psum_tensor("s1", [128, 2 * QC], f32))
        s2 = E(nc.psum_tensor("s2", [128, 2 * QC], f32))
        po_both = E(nc.psum_tensor("po_both", [128, QC], f32))
        pq0 = E(nc.psum_tensor("pq0", [128, QC], f32))
        dIN = E(nc.semaphore("dIN"))
        mset = E(nc.semaphore("mset"))
        sPQ = E(nc.semaphore("sPQ"))
        sCP = E(nc.semaphore("sCP"))  # DVE production copies
        sCA = E(nc.semaphore("sCA"))  # ACT production copies
        sSC = E(nc.semaphore("sSC"))
        sEXA = E(nc.semaphore("sEXA"))
        sEXD = E(nc.semaphore("sEXD"))
        sAV = E(nc.semaphore("sAV"))
        sPOC = E(nc.semaphore("sPOC"))
        sRC = E(nc.semaphore("sRC"))
        sMU = E(nc.semaphore("sMU"))
        sPJ = E(nc.semaphore("sPJ"))
        sOS = E(nc.semaphore("sOS"))
        sOD0 = E(nc.semaphore("sOD0"))
        sOD1 = E(nc.semaphore("sOD1"))
        sF8K = E(nc.semaphore("sF8K"))
        sF8Q = E(nc.semaphore("sF8Q"))
        block = E(nc.Block())

        s_ = (s0, s1, s2)
        pt_ = (pt0, pt1, pt2, pt3)
        pt_ap = tuple(t.ap() for t in pt_)
        po_sb_ = (po_sb0, po_sb1)
        osb_ = (osb0, osb1)
        oT_ = (oT0, oT1)

        w = wpk.ap()
        twq0, twk0, twv0 = w[:, 0:128], w[:, 128:256], w[:, 256:320]
        twq1, twk1, twv1 = w[:, 320:448], w[:, 448:576], w[:, 576:640]
        twqb, twkb, twvb = w[0:1, 640:768], w[0:1, 768:896], w[0:1, 896:960]
        twp0, twp1 = w[0:Dh, 960:1216], w[0:Dh, 1216:1472]

        pob = po_both.ap()

        def po_av(i):  # AV rows of the po ping-pong half
            return pob[0:Dh, :] if i % 2 == 0 else pob[64 : 64 + Dh, :]

        def po_full(i):
            return pob[0 : 2 * Dh, :] if i % 2 == 0 else pob[64:128, :]

        def po_den(i):
            return pob[Dh : 2 * Dh, :] if i % 2 == 0 else pob[64 + Dh : 128, :]

        # prologue projection slots (all bank starts)
        sa = (s0.ap(), s1.ap(), s2.ap())
        v_slots = (
            pq0.ap(), pob, sa[0], sa[0][:, QC:], sa[1], sa[1][:, QC:],
            sa[2], sa[2][:, QC:],
        )
        qk_slots = (pq0.ap(), pob, sa[0], sa[1])

        def kc_of(t):  # k chunk needed by score tile t
            return (2 * t + 1) // 4

        def wait_exp(eng, g):
            """Wait until exp(g) is done (engine-specific counter)."""
            if g < 0:
                return
            if asg[g] == 0:
                eng.wait_ge(sEXA, preA[g] + 1)
            else:
                eng.wait_ge(sEXD, preD[g] + 1)

        @block.sync
        def _(sync):
            sync.dma_start(out=wpk[:, :], in_=ins["wpack"].ap()).then_inc(dIN, 16)
            for j in range(NXC):  # x chunks, interleaved halves
                sl = slice(j * XC, (j + 1) * XC)
                sync.dma_start(out=xt0[:, sl], in_=ins["x0"].ap()[:, sl]).then_inc(
                    dIN, 16
                )
                sync.dma_start(out=xt1[:, sl], in_=ins["x1"].ap()[:, sl]).then_inc(
                    dIN, 16
                )
            for k in range(2 * NQC):  # out DMAs: c-major, mc-minor
                c, mc = k // 2, k % 2
                sync.wait_ge(sOS, k + 1)
                sync.dma_start(
                    out=out.ap()[mc * 128 : (mc + 1) * 128, c * QC : (c + 1) * QC],
                    in_=osb_[k % 2][:, :],
                ).then_inc((sOD0, sOD1)[k % 2], 16)

        @block.gpsimd
        def _(gpsimd):
            gpsimd.memset(ones_sm[:, :], 1.0).then_inc(mset, 1)
            # only the ones-halves; the v-halves are written by the copies
            gpsimd.memset(v_all[:, :, :, Dh : 2 * Dh], 1.0).then_inc(mset, 1)
            # fp8 pair-packed copies from the f32r tensors (k first: DR tiles
            # of iteration IDR need all k chunks early).  The source rows
            # include the zero-padding rows, so no fp8 memsets are needed.
            for c in range(NQC):
                sl = slice(c * QC, (c + 1) * QC)
                gpsimd.wait_ge(sCP, 24 + c + 1)  # kT chunk c copied
                for h in (0, 1):
                    for j_ in (0, 1):
                        gpsimd.tensor_copy(
                            kd8.ap()[32 * h : 32 * h + 32, j_, sl],
                            kT.ap()[64 * h + 32 * j_ : 64 * h + 32 * j_ + 32, sl]
                            .bitcast(f32),
                        ).then_inc(sF8K, 1)
            for c in range(NQC):
                sl = slice(c * QC, (c + 1) * QC)
                gpsimd.wait_ge(sCA, 8 + c + 1)  # qT chunk c copied
                for h in (0, 1):
                    for j_ in (0, 1):
                        gpsimd.tensor_copy(
                            qd8.ap()[32 * h : 32 * h + 32, j_, sl],
                            qT.ap()[64 * h + 32 * j_ : 64 * h + 32 * j_ + 32, sl]
                            .bitcast(f32),
                        ).then_inc(sF8Q, 1)

        @block.tensor
        def _(tensor):
            tensor.wait_ge(mset, 1)

            # ---- v projection first: 32 blocks over 8 slots, chasing DMA ----
            for pb in range(NKB):
                psl = slice(pb * KB, (pb + 1) * KB)
                tensor.wait_ge(dIN, 16 + 32 * (pb // 8 + 1))
                if pb >= 8:  # slot pb-8's copy done
                    pr_ = pb - 8
                    if pr_ % 4 == 3:
                        tensor.wait_ge(sCA, pr_ // 4 + 1)
                    else:
                        tensor.wait_ge(sCP, pr_ + 1 - (pr_ + 1) // 4)
                pv = v_slots[pb % 8][0:128, 0:64]
                tensor.matmul(pv, xt0[:, psl], twv0, start=True, stop=False)
                tensor.matmul(pv, xt1[:, psl], twv1, start=False, stop=False)
                tensor.matmul(
                    pv, ones_sm[0:1, 0:KB], twvb, start=False, stop=True
                ).then_inc(sPQ, 1)

            # ---- q/k projections: 16 groups, c-major, k then q ----
            for g in range(2 * NQC):
                c, which = g // 2, g % 2  # which: 0 = k, 1 = q
                sl = slice(c * QC, (c + 1) * QC)
                w0, w1, wb = ((twk0, twk1, twkb), (twq0, twq1, twqb))[which]
                if g < 4:  # slot still held by the v phase
                    tensor.wait_ge(sCP, 24)
                    tensor.wait_ge(sCA, 8)
                else:
                    base = (24, 8)[(g - 4) % 2]
                    tensor.wait_ge((sCP, sCA)[(g - 4) % 2], base + (g - 4) // 2 + 1)
                p = qk_slots[g % 4][0:128, 0:QC]
                tensor.matmul(p, w0, xt0[:, sl], start=True, stop=False)
                tensor.matmul(p, w1, xt1[:, sl], start=False, stop=False)
                tensor.matmul(
                    p, wb, ones_sm[:, :], start=False, stop=True
                ).then_inc(sPQ, 1)

            def scores(g):
                i, t = g // NT, g % NT
                c, h = i // 2, i % 2
                hsl = slice(64 * h, 64 * h + 64)
                h32 = slice(32 * h, 32 * h + 32)
                qsl = slice(c * QC, (c + 1) * QC)
                use_dr = i >= IDR
                if i == 0:
                    tensor.wait_ge(sCP, 25 + max(kc_of(t), 1 if t == 0 else 0))
                    tensor.wait_ge(sCA, 10 if t == 1 else 9)
                if use_dr:
                    # fp8 copies for the needed k chunk + q chunk ready
                    tensor.wait_ge(sF8K, 4 * (kc_of(t) + 1))
                    tensor.wait_ge(sF8Q, 4 * (c + 1))
                wait_exp(tensor, g - 3)  # s[g%3] free after exp g-3
                for j in (0, 1):
                    kb = 2 * t + j
                    ksl = slice(kb * KB, (kb + 1) * KB)
                    if use_dr:
                        mm = tensor.matmul(
                            s_[g % 3][:, j * QC : (j + 1) * QC],
                            kd8.ap()[h32, :, ksl],
                            qd8.ap()[h32, :, qsl],
                            start=True,
                            stop=True,
                            perf_mode=mybir.MatmulPerfMode.DoubleRow,
                        )
                    else:
                        mm = tensor.matmul(
                            s_[g % 3][:, j * QC : (j + 1) * QC],
                            kT[hsl, ksl],
                            qT[hsl, qsl],
                            start=True,
                            stop=True,
                        )
                    if j == 1:
                        mm.then_inc(sSC, 1)

            def av(g):
                i, t = g // NT, g % NT
                h = i % 2
                wait_exp(tensor, g)
                if g == 0:  # po/pq banks still held by qk-copy readers
                    tensor.wait_ge(sCP, 32)
                    tensor.wait_ge(sCA, 16)
                    tensor.wait_ge(mset, 2)  # v_all ones columns ready
                if t == 0 and i >= 2:
                    tensor.wait_ge(sPOC, i - 1)  # po half free (ACT copy done)
                    tensor.wait_ge(sRC, i - 1)  # and recip done
                for j in (0, 1):
                    kb = 2 * t + j
                    mm = tensor.matmul(
                        po_full(i),
                        v_all[:, kb, h, :],
                        pt_ap[g % 4][:, j * QC : (j + 1) * QC],
                        start=(kb == 0),
                        stop=(kb == NKB - 1),
                        skip_group_check=True,
                    )
                    if j == 1:
                        mm.then_inc(sAV, 1)

            def proj_mc(c, mc):
                qsl = slice(c * QC, (c + 1) * QC)
                if mc == 0:
                    tensor.wait_ge(sMU, 2 * c + 2)
                k = 2 * c + mc
                msl = slice(mc * 128, (mc + 1) * 128)
                if k == 2 * NQC - 1:
                    dst = pob[0:128, 0:QC]  # po retired; skip the osb wait
                else:
                    if k >= 1:
                        tensor.wait_ge(sOS, k)  # pq0 free after osb copy k-1
                    dst = pq0[:, :]
                tensor.matmul(
                    dst, twp0[:, msl], oT0[:, qsl], start=True, stop=False
                )
                tensor.matmul(
                    dst, twp1[:, msl], oT1[:, qsl], start=False, stop=True
                ).then_inc(sPJ, 1)

            for g in range(NG):
                scores(g)
                if g >= 2:
                    av(g - 2)
                    j = g - 2
                    if j % NT == NT - 1:
                        i_done = j // NT
                        if i_done >= 2 and i_done % 2 == 0:
                            proj_mc((i_done - 2) // 2, 0)
                    if j % NT == 1:
                        i = j // NT
                        if i >= 3 and i % 2 == 1:
                            proj_mc((i - 3) // 2, 1)
            av(NG - 2)
            av(NG - 1)
            proj_mc(NQC - 1, 0)
            proj_mc(NQC - 1, 1)

        @block.scalar
        def _(scalar):
            # ACT: every 4th v copy + q copies (f32r), then exp + po copies.
            for pb in range(3, NKB, 4):
                scalar.wait_ge(sPQ, pb + 1)
                pv = v_slots[pb % 8][0:128, 0:64]
                scalar.copy(
                    v_all.ap()[:, pb, :, 0:Dh], pv[:, 0 : 2 * Dh]
                ).then_inc(sCA, 1)
            def q_copy(c):
                g = 2 * c + 1
                sl = slice(c * QC, (c + 1) * QC)
                scalar.wait_ge(sPQ, NKB + g + 1)
                scalar.copy(
                    qT[:, sl], qk_slots[g % 4][0:128, 0:QC]
                ).then_inc(sCA, 1)

            for c_ in range(NQC):
                q_copy(c_)

            def po_copy(i):
                scalar.wait_ge(sAV, NT * (i + 1))
                if i >= 2:
                    scalar.wait_ge(sMU, i - 1)  # po_sb[i%2] reader done
                scalar.copy(po_sb_[i % 2][:, :], po_av(i)).then_inc(sPOC, 1)

            def osb_copy_act(k):
                scalar.wait_ge(sPJ, k + 1)
                src_ = pob[0:128, 0:QC] if k == 2 * NQC - 1 else pq0[:, :]
                scalar.copy(osb_[k % 2][:, :], src_).then_inc(sOS, 1)

            pending = -1  # iteration whose po_copy is pending
            for g in range(NG):
                i, t = g // NT, g % NT
                if asg[g] == 0:
                    na = preA[g + 1] - preA[i * NT]  # ACT-tile index within iter
                    scalar.wait_ge(sSC, g + 1)
                    if g >= 4:
                        scalar.wait_ge(sAV, g - 3)  # pt[g%4] free after av g-4
                    scalar.activation(
                        pt_[g % 4][:, :], s_[g % 3][:, :], EXP, 0.0, LN2
                    ).then_inc(sEXA, 1)
                    if i >= 1 and na == 4 and pending == i - 1:
                        po_copy(i - 1)
                        pending = -2
                if t == NT - 1:
                    if pending >= 0:
                        po_copy(pending)
                    pending = i
            po_copy(NI - 1)
            osb_copy_act(2 * NQC - 1)

        @block.vector
        def _(vector):
            # three of every four v copies, then k copies (f32r)
            for pb in range(NKB):
                if pb % 4 == 3:
                    continue
                vector.wait_ge(sPQ, pb + 1)
                pv = v_slots[pb % 8][0:128, 0:64]
                vector.tensor_copy(
                    v_all.ap()[:, pb, :, 0:Dh], pv[:, 0 : 2 * Dh]
                ).then_inc(sCP, 1)
            def k_copy(c):
                g = 2 * c
                sl = slice(c * QC, (c + 1) * QC)
                vector.wait_ge(sPQ, NKB + g + 1)
                vector.tensor_copy(
                    kT[:, sl], qk_slots[g % 4][0:128, 0:QC]
                ).then_inc(sCP, 1)

            for c_ in range(NQC):
                k_copy(c_)

            def exp_dve(g):
                vector.wait_ge(sSC, g + 1)
                if g >= 4:
                    vector.wait_ge(sAV, g - 3)
                vector.tensor_scalar(
                    pt_ap[g % 4][:, :].bitcast(i16),
                    s_[g % 3][:, :],
                    128.0,
                    16256.0 + SIGMA,
                    Alu.mult,
                    Alu.add,
                ).then_inc(sEXD, 1)

            def epi_recip(i):
                vector.wait_ge(sAV, NT * (i + 1))
                vector.reciprocal(rsb[:, :], po_den(i)).then_inc(sRC, 1)

            def epi_mul(i):
                h = i % 2
                c = i // 2
                qsl = slice(c * QC, (c + 1) * QC)
                vector.wait_ge(sPOC, i + 1)
                vector.tensor_mul(
                    oT_[h][:, qsl], po_sb_[i % 2][:, :], rsb[:, :]
                ).then_inc(sMU, 1)

            def epilogue(i):
                epi_recip(i)
                epi_mul(i)

            def osb_copy(k):
                vector.wait_ge(sPJ, k + 1)
                if k >= 2:
                    vector.wait_ge((sOD0, sOD1)[k % 2], 16 * (k // 2))
                src_ = pob[0:128, 0:QC] if k == 2 * NQC - 1 else pq0[:, :]
                vector.tensor_copy(osb_[k % 2][:, :], src_).then_inc(sOS, 1)

            for i in range(NI):
                for t in range(NT):
                    g = i * NT + t
                    if asg[g] == 1:
                        exp_dve(g)
                        nd = preD[g + 1] - preD[i * NT]
                        if i >= 1 and nd == 1:
                            epi_recip(i - 1)
                        if i >= 1 and nd == 2:
                            epi_mul(i - 1)
                        if i >= 3 and i % 2 == 1 and nd == 3:
                            osb_copy(i - 2)
                # osb copy for proj mc=0 lands right after this iteration's
                # last DVE exp so the PE's next sOS wait can resolve
                if i >= 2 and i % 2 == 0:
                    osb_copy(i - 2)
            epilogue(NI - 1)
            osb_copy(2 * NQC - 2)

    return nc


def _prep_in_maps(x, W_qkv, b_qkv, W_proj):
    import ml_dtypes

    bf = ml_dtypes.bfloat16
    in_maps = []
    for i in range(NCORES):
        b = i // 4
        heads = [2 * (i % 4), 2 * (i % 4) + 1]
        xT = np.ascontiguousarray(x[b].T.astype(np.float32))  # [256, 4096]

        def slc(base, scale):
            w = np.concatenate(
                [W_qkv[:, base + h * Dh : base + (h + 1) * Dh] for h in heads], axis=1
            ).astype(np.float32) * scale
            bb = np.concatenate(
                [b_qkv[base + h * Dh : base + (h + 1) * Dh] for h in heads]
            ).astype(np.float32)[None, :] * scale
            return w, bb

        wq, bq = slc(0, SCALE * LOG2E)
        wk, bk = slc(C, 1.0)
        wv, bv = slc(2 * C, 1.0)
        wp = np.concatenate(
            [W_proj[h * Dh : (h + 1) * Dh, :] for h in heads], axis=0
        ).astype(np.float32)  # [64, 256]

        def pad_qk(w64):
            # [rows, 64] -> [rows, 128]: head h dim d=2p+j -> col 64h+32j+p,
            # cols 64h+32j+16 .. +32 stay zero (DoubleRow pair padding)
            o = np.zeros((w64.shape[0], 128), dtype=np.float32)
            for h in (0, 1):
                wh = w64[:, 32 * h : 32 * h + 32]
                for j in (0, 1):
                    o[:, 64 * h + 32 * j : 64 * h + 32 * j + 16] = wh[:, j::2]
            return o

        wqp, wkp = pad_qk(wq), pad_qk(wk)
        bqp, bkp = pad_qk(bq), pad_qk(bk)

        wpack = np.zeros((128, WPK), dtype=np.float32)
        wpack[:, 0:128] = wqp[:128]
        wpack[:, 128:256] = wkp[:128]
        wpack[:, 256:320] = wv[:128]
        wpack[:, 320:448] = wqp[128:]
        wpack[:, 448:576] = wkp[128:]
        wpack[:, 576:640] = wv[128:]
        wpack[0, 640:768] = bqp[0]
        wpack[0, 768:896] = bkp[0]
        wpack[0, 896:960] = bv[0]
        wpack[0:Dh, 960:1216] = wp[:Dh]
        wpack[0:Dh, 1216:1472] = wp[Dh:]

        m = {
            "wpack": wpack.astype(bf),
            "x0": xT[:128].astype(bf),
            "x1": xT[128:].astype(bf),
        }
        in_maps.append({k: np.ascontiguousarray(v) for k, v in m.items()})
    return in_maps


LAST_RESULT = None


def kernel(x, W_qkv, b_qkv, W_proj, b_proj):
    global LAST_RESULT
    from concourse.bass_utils import run_bass_kernel_spmd

    if "nc" not in _CACHE:
        _CACHE["nc"] = _build_nc()
    nc = _CACHE["nc"]

    in_maps = _prep_in_maps(
        np.asarray(x), np.asarray(W_qkv), np.asarray(b_qkv), np.asarray(W_proj)
    )
    res = run_bass_kernel_spmd(nc, in_maps, core_ids=list(range(NCORES)))
    LAST_RESULT = res
    outs = res.results
    full = np.zeros((B, N, C), dtype=np.float32)
    for i in range(NCORES):
        b = i // 4
        full[b] += np.asarray(outs[i]["out"]).T
    full += np.asarray(b_proj).astype(np.float32)[None, None, :]
    return full


# revision 33
# speedup vs baseline: 1.0933x; 1.0642x over previous
"""Multi-head attention kernel for 8 TRN2 NeuronCores (raw Bass, no Tile).

Problem: x[2,4096,256] -> qkv proj -> 8-head attention (Dh=32) -> out proj.
Sharding: 16 (batch, head) pairs over 8 cores: core i handles batch i//4 and
heads {2*(i%4), 2*(i%4)+1}. Each core computes its 2 heads' attention plus the
partial output projection; host sums the 4 partial projections per batch.

Per-core design:
  - qkv projections in bf16 (x and weights host-cast; all weights arrive in
    one packed DMA; bias via ones-row augmented matmuls).  q is pre-scaled by
    SCALE*log2(e) host-side so scores come out as z = log2(e)*s.
  - v projected first (8 bank-aligned PSUM slots, copies split ACT/DVE, one
    strided 3-D copy per k-block), then q/k (4 slots, k-copies on DVE,
    q-copies on ACT as activation-Copy with float32r output).
  - scores in float32r (the copies provide the BIR-required f32r rounding):
    S^T[128 kpos, 512 q] tiles at 1 cycle/row.
  - exp split across two engines (the throughput bottleneck):
      ACT: exact exp via activation(Exp, scale=ln2) -> bf16
      DVE: Schraudolph bit-trick: int16(z*128 + 16251.25) bitcast to bf16
           (exact round-to-nearest on HW; softmax normalization cancels most
           of the +-4% sawtooth)
  - AV in bf16 with a packed v [v_h(32)|ones(32)] per (k-block, head) so the
    matmul both accumulates AV and broadcasts the softmax denominator:
    po[64, 512] = rows 0:32 AV, rows 32:64 denominator (replicated).
  - epilogue: DVE reciprocal(den) + ACT copy(AV) + DVE SBUF mul -> oT bf16.
  - out proj in bf16 from oT; partials DMA'd as f32 [256, 4096].

Pipeline: 3 score PSUM slots + AV lagging scores by 2 tiles keep the PE from
round-tripping on exp latency; 4 exp SBUF slots decouple the exp engines; the
two out-proj halves straddle an iteration boundary so the PSUM->SBUF copy
between them hides under attention tiles.
PSUM banks: s0,s1,s2 (2 each) + po_both (1; halves are the po ping-pong) +
pq0 (1).  The prologue reuses all of them as projection slots.
"""

import sys

sys.path.insert(0, "/opt/trn_rl_repo")

import numpy as np

B, N, C, H, Dh = 2, 4096, 256, 8, 32
HPC = 2  # heads per core
NCORES = 8
SCALE = C ** -0.5
LOG2E = float(np.log2(np.e))
LN2 = float(np.log(2.0))
SIGMA = -4.75  # Schraudolph mean-centering shift
QC = 512  # q columns per chunk
NQC = N // QC  # 8
KB = 128  # k rows per block
NKB = N // KB  # 32
NI = NQC * HPC  # 16 (c, h) iterations
NT = NKB // 2  # 16 double-kb tiles per iteration
NG = NI * NT  # 256 score/exp/AV tiles
WPK = 1472  # packed weight tensor columns
IDR = 2  # first iteration using fp8 DoubleRow for odd-t score tiles
XC = 1024  # x DMA chunk columns
NXC = N // XC  # 4 chunk pairs

# exp tile assignment: per-iteration t values handled by DVE (Schraudolph);
# the rest go to ACT (exact exp).
DVE_T = (2, 5, 7, 9, 11, 13, 15)

_CACHE = {}


def _build_nc():
    import concourse.bass as bass
    import concourse.mybir as mybir

    f32 = mybir.dt.float32
    f32r = mybir.dt.float32r
    bf16 = mybir.dt.bfloat16
    i16 = mybir.dt.int16
    EXP = mybir.ActivationFunctionType.Exp
    Alu = mybir.AluOpType

    # exp engine assignment tables
    asg = [1 if (g % NT) in DVE_T else 0 for g in range(NG)]  # 1 = DVE
    # tail chain: last exp on ACT (the po copy follows there)
    asg[NG - 1] = 0
    preA = [0] * (NG + 1)
    preD = [0] * (NG + 1)
    for g in range(NG):
        preA[g + 1] = preA[g] + (asg[g] == 0)
        preD[g + 1] = preD[g] + (asg[g] == 1)

    nc = bass.Bass("TRN2", target_bir_lowering=False, debug=False, num_devices=NCORES)

    ins = {}
    for nm, shp, dt_ in (
        ("wpack", [128, WPK], bf16),
        ("x0", [128, N], bf16),
        ("x1", [128, N], bf16),
    ):
        ins[nm] = nc.dram_tensor(nm, shp, dt_, kind="ExternalInput")
    out = nc.dram_tensor("out", [C, N], f32, kind="ExternalOutput")

    from contextlib import ExitStack
    with ExitStack() as ctx:
        E = ctx.enter_context
        xt0 = E(nc.sbuf_tensor("xt0", [128, N], bf16))
        xt1 = E(nc.sbuf_tensor("xt1", [128, N], bf16))
        wpk = E(nc.sbuf_tensor("wpk", [128, WPK], bf16))
        ones_sm = E(nc.sbuf_tensor("ones_sm", [1, QC], bf16))
        qT = E(nc.sbuf_tensor("qT", [128, N], f32r))
        kT = E(nc.sbuf_tensor("kT", [128, N], f32r))
        f8 = mybir.dt.float8e4
        qd8 = E(nc.sbuf_tensor("qd8", [64, 2, N], f8))
        kd8 = E(nc.sbuf_tensor("kd8", [64, 2, N], f8))
        v_all = E(nc.sbuf_tensor("v_all", [128, NKB, 2, 2 * Dh], bf16))
        pt0 = E(nc.sbuf_tensor("pt0", [128, 2 * QC], bf16))
        pt1 = E(nc.sbuf_tensor("pt1", [128, 2 * QC], bf16))
        pt2 = E(nc.sbuf_tensor("pt2", [128, 2 * QC], bf16))
        pt3 = E(nc.sbuf_tensor("pt3", [128, 2 * QC], bf16))
        oT0 = E(nc.sbuf_tensor("oT0", [Dh, N], bf16))
        oT1 = E(nc.sbuf_tensor("oT1", [Dh, N], bf16))
        po_sb0 = E(nc.sbuf_tensor("po_sb0", [Dh, QC], f32))
        po_sb1 = E(nc.sbuf_tensor("po_sb1", [Dh, QC], f32))
        rsb = E(nc.sbuf_tensor("rsb", [Dh, QC], f32))
        osb0 = E(nc.sbuf_tensor("osb0", [128, QC], f32))
        osb1 = E(nc.sbuf_tensor("osb1", [128, QC], f32))
        s0 = E(nc.psum_tensor("s0", [128, 2 * QC], f32))
        s1 = E(nc.psum_tensor("s1", [128, 2 * QC], f32))
        s2 = E(nc.psum_tensor("s2", [128, 2 * QC], f32))
        po_both = E(nc.psum_tensor("po_both", [128, QC], f32))
        pq0 = E(nc.psum_tensor("pq0", [128, QC], f32))
        dIN = E(nc.semaphore("dIN"))
        mset = E(nc.semaphore("mset"))
        sPQ = E(nc.semaphore("sPQ"))
        sCP = E(nc.semaphore("sCP"))  # DVE production copies
        sCA = E(nc.semaphore("sCA"))  # ACT production copies
        sSC = E(nc.semaphore("sSC"))
        sEXA = E(nc.semaphore("sEXA"))
        sEXD = E(nc.semaphore("sEXD"))
        sAV = E(nc.semaphore("sAV"))
        sPOC = E(nc.semaphore("sPOC"))
        sRC = E(nc.semaphore("sRC"))
        sMU = E(nc.semaphore("sMU"))
        sPJ = E(nc.semaphore("sPJ"))
        sOS = E(nc.semaphore("sOS"))
        sOD0 = E(nc.semaphore("sOD0"))
        sOD1 = E(nc.semaphore("sOD1"))
        sF8K = E(nc.semaphore("sF8K"))
        sF8Q = E(nc.semaphore("sF8Q"))
        block = E(nc.Block())

        s_ = (s0, s1, s2)
        pt_ = (pt0, pt1, pt2, pt3)
        pt_ap = tuple(t.ap() for t in pt_)
        po_sb_ = (po_sb0, po_sb1)
        osb_ = (osb0, osb1)
        oT_ = (oT0, oT1)

        w = wpk.ap()
        twq0, twk0, twv0 = w[:, 0:128], w[:, 128:256], w[:, 256:320]
        twq1, twk1, twv1 = w[:, 320:448], w[:, 448:576], w[:, 576:640]
        twqb, twkb, twvb = w[0:1, 640:768], w[0:1, 768:896], w[0:1, 896:960]
        twp0, twp1 = w[0:Dh, 960:1216], w[0:Dh, 1216:1472]

        pob = po_both.ap()

        def po_av(i):  # AV rows of the po ping-pong half
            return pob[0:Dh, :] if i % 2 == 0 else pob[64 : 64 + Dh, :]

        def po_full(i):
            return pob[0 : 2 * Dh, :] if i % 2 == 0 else pob[64:128, :]

        def po_den(i):
            return pob[Dh : 2 * Dh, :] if i % 2 == 0 else pob[64 + Dh : 128, :]

        # prologue projection slots (all bank starts)
        sa = (s0.ap(), s1.ap(), s2.ap())
        v_slots = (
            pq0.ap(), pob, sa[0], sa[0][:, QC:], sa[1], sa[1][:, QC:],
            sa[2], sa[2][:, QC:],
        )
        qk_slots = (pq0.ap(), pob, sa[0], sa[1])

        def kc_of(t):  # k chunk needed by score tile t
            return (2 * t + 1) // 4

        def wait_exp(eng, g):
            """Wait until exp(g) is done (engine-specific counter)."""
            if g < 0:
                return
            if asg[g] == 0:
                eng.wait_ge(sEXA, preA[g] + 1)
            else:
                eng.wait_ge(sEXD, preD[g] + 1)

        @block.sync
        def _(sync):
            sync.dma_start(out=wpk[:, :], in_=ins["wpack"].ap()).then_inc(dIN, 16)
            for j in range(NXC):  # x chunks, interleaved halves
                sl = slice(j * XC, (j + 1) * XC)
                sync.dma_start(out=xt0[:, sl], in_=ins["x0"].ap()[:, sl]).then_inc(
                    dIN, 16
                )
                sync.dma_start(out=xt1[:, sl], in_=ins["x1"].ap()[:, sl]).then_inc(
                    dIN, 16
                )
            for k in range(2 * NQC):  # out DMAs: c-major, mc-minor
                c, mc = k // 2, k % 2
                sync.wait_ge(sOS, k + 1)
                sync.dma_start(
                    out=out.ap()[mc * 128 : (mc + 1) * 128, c * QC : (c + 1) * QC],
                    in_=osb_[k % 2][:, :],
                ).then_inc((sOD0, sOD1)[k % 2], 16)

        @block.gpsimd
        def _(gpsimd):
            gpsimd.memset(ones_sm[:, :], 1.0).then_inc(mset, 1)
            # only the ones-halves; the v-halves are written by the copies
            gpsimd.memset(v_all[:, :, :, Dh : 2 * Dh], 1.0).then_inc(mset, 1)
            # fp8 pair-packed copies from the f32r tensors (k first: DR tiles
            # of iteration IDR need all k chunks early).  The source rows
            # include the zero-padding rows, so no fp8 memsets are needed.
            for c in range(NQC):
                sl = slice(c * QC, (c + 1) * QC)
                gpsimd.wait_ge(sCP, 24 + c + 1)  # kT chunk c copied
                for h in (0, 1):
                    for j_ in (0, 1):
                        gpsimd.tensor_copy(
                            kd8.ap()[32 * h : 32 * h + 32, j_, sl],
                            kT.ap()[64 * h + 32 * j_ : 64 * h + 32 * j_ + 32, sl]
                            .bitcast(f32),
                        ).then_inc(sF8K, 1)
            for c in range(NQC):
                sl = slice(c * QC, (c + 1) * QC)
                gpsimd.wait_ge(sCA, 8 + c + 1)  # qT chunk c copied
                for h in (0, 1):
                    for j_ in (0, 1):
                        gpsimd.tensor_copy(
                            qd8.ap()[32 * h : 32 * h + 32, j_, sl],
                            qT.ap()[64 * h + 32 * j_ : 64 * h + 32 * j_ + 32, sl]
                            .bitcast(f32),
                        ).then_inc(sF8Q, 1)

        @block.tensor
        def _(tensor):
            tensor.wait_ge(mset, 1)

            # ---- v projection first: 32 blocks over 8 slots, chasing DMA ----
            for pb in range(NKB):
                psl = slice(pb * KB, (pb + 1) * KB)
                tensor.wait_ge(dIN, 16 + 32 * (pb // 8 + 1))
                if pb >= 8:  # slot pb-8's copy done
                    pr_ = pb - 8
                    if pr_ % 4 == 3:
                        tensor.wait_ge(sCA, pr_ // 4 + 1)
                    else:
                        tensor.wait_ge(sCP, pr_ + 1 - (pr_ + 1) // 4)
                pv = v_slots[pb % 8][0:128, 0:64]
                tensor.matmul(pv, xt0[:, psl], twv0, start=True, stop=False)
                tensor.matmul(pv, xt1[:, psl], twv1, start=False, stop=False)
                tensor.matmul(
                    pv, ones_sm[0:1, 0:KB], twvb, start=False, stop=True
                ).then_inc(sPQ, 1)

            # ---- q/k projections: 16 groups, c-major, k then q ----
            for g in range(2 * NQC):
                c, which = g // 2, g % 2  # which: 0 = k, 1 = q
                sl = slice(c * QC, (c + 1) * QC)
                w0, w1, wb = ((twk0, twk1, twkb), (twq0, twq1, twqb))[which]
                if g < 4:  # slot still held by the v phase
                    tensor.wait_ge(sCP, 24)
                    tensor.wait_ge(sCA, 8)
                else:
                    base = (24, 8)[(g - 4) % 2]
                    tensor.wait_ge((sCP, sCA)[(g - 4) % 2], base + (g - 4) // 2 + 1)
                p = qk_slots[g % 4][0:128, 0:QC]
                tensor.matmul(p, w0, xt0[:, sl], start=True, stop=False)
                tensor.matmul(p, w1, xt1[:, sl], start=False, stop=False)
                tensor.matmul(
                    p, wb, ones_sm[:, :], start=False, stop=True
                ).then_inc(sPQ, 1)

            def scores(g):
                i, t = g // NT, g % NT
                c, h = i // 2, i % 2
                hsl = slice(64 * h, 64 * h + 64)
                h32 = slice(32 * h, 32 * h + 32)
                qsl = slice(c * QC, (c + 1) * QC)
                use_dr = i >= IDR
                if i == 0:
                    tensor.wait_ge(sCP, 25 + max(kc_of(t), 1 if t == 0 else 0))
                    tensor.wait_ge(sCA, 10 if t == 1 else 9)
                if use_dr:
                    # fp8 copies for the needed k chunk + q chunk ready
                    tensor.wait_ge(sF8K, 4 * (kc_of(t) + 1))
                    tensor.wait_ge(sF8Q, 4 * (c + 1))
                wait_exp(tensor, g - 3)  # s[g%3] free after exp g-3
                for j in (0, 1):
                    kb = 2 * t + j
                    ksl = slice(kb * KB, (kb + 1) * KB)
                    if use_dr:
                        mm = tensor.matmul(
                            s_[g % 3][:, j * QC : (j + 1) * QC],
                            kd8.ap()[h32, :, ksl],
                            qd8.ap()[h32, :, qsl],
                            start=True,
                            stop=True,
                            perf_mode=mybir.MatmulPerfMode.DoubleRow,
                        )
                    else:
                        mm = tensor.matmul(
                            s_[g % 3][:, j * QC : (j + 1) * QC],
                            kT[hsl, ksl],
                            qT[hsl, qsl],
                            start=True,
                            stop=True,
                        )
                    if j == 1:
                        mm.then_inc(sSC, 1)

            def av(g):
                i, t = g // NT, g % NT
                h = i % 2
                wait_exp(tensor, g)
                if g == 0:  # po/pq banks still held by qk-copy readers
                    tensor.wait_ge(sCP, 32)
                    tensor.wait_ge(sCA, 16)
                    tensor.wait_ge(mset, 2)  # v_all ones columns ready
                if t == 0 and i >= 2:
                    tensor.wait_ge(sPOC, i - 1)  # po half free (ACT copy done)
                    tensor.wait_ge(sRC, i - 1)  # and recip done
                for j in (0, 1):
                    kb = 2 * t + j
                    mm = tensor.matmul(
                        po_full(i),
                        v_all[:, kb, h, :],
                        pt_ap[g % 4][:, j * QC : (j + 1) * QC],
                        start=(kb == 0),
                        stop=(kb == NKB - 1),
                        skip_group_check=True,
                    )
                    if j == 1:
                        mm.then_inc(sAV, 1)

            def proj_mc(c, mc):
                qsl = slice(c * QC, (c + 1) * QC)
                if mc == 0:
                    tensor.wait_ge(sMU, 2 * c + 2)
                k = 2 * c + mc
                msl = slice(mc * 128, (mc + 1) * 128)
                if k == 2 * NQC - 1:
                    dst = pob[0:128, 0:QC]  # po retired; skip the osb wait
                else:
                    if k >= 1:
                        tensor.wait_ge(sOS, k)  # pq0 free after osb copy k-1
                    dst = pq0[:, :]
                tensor.matmul(
                    dst, twp0[:, msl], oT0[:, qsl], start=True, stop=False
                )
                tensor.matmul(
                    dst, twp1[:, msl], oT1[:, qsl], start=False, stop=True
                ).then_inc(sPJ, 1)

            for g in range(NG):
                scores(g)
                if g >= 3:
                    av(g - 3)
                    j = g - 3
                    if j % NT == NT - 1:
                        i_done = j // NT
                        if i_done >= 2 and i_done % 2 == 0:
                            proj_mc((i_done - 2) // 2, 0)
                    if j % NT == 1:
                        i = j // NT
                        if i >= 3 and i % 2 == 1:
                            proj_mc((i - 3) // 2, 1)
            av(NG - 3)
            av(NG - 2)
            av(NG - 1)
            proj_mc(NQC - 1, 0)
            proj_mc(NQC - 1, 1)

        @block.scalar
        def _(scalar):
            # ACT: every 4th v copy + q copies (f32r), then exp + po copies.
            for pb in range(3, NKB, 4):
                scalar.wait_ge(sPQ, pb + 1)
                pv = v_slots[pb % 8][0:128, 0:64]
                scalar.copy(
                    v_all.ap()[:, pb, :, 0:Dh], pv[:, 0 : 2 * Dh]
                ).then_inc(sCA, 1)
            def q_copy(c):
                g = 2 * c + 1
                sl = slice(c * QC, (c + 1) * QC)
                scalar.wait_ge(sPQ, NKB + g + 1)
                scalar.copy(
                    qT[:, sl], qk_slots[g % 4][0:128, 0:QC]
                ).then_inc(sCA, 1)

            for c_ in range(NQC):
                q_copy(c_)

            def po_copy(i):
                scalar.wait_ge(sAV, NT * (i + 1))
                if i >= 2:
                    scalar.wait_ge(sMU, i - 1)  # po_sb[i%2] reader done
                scalar.copy(po_sb_[i % 2][:, :], po_av(i)).then_inc(sPOC, 1)

            def osb_copy_act(k):
                scalar.wait_ge(sPJ, k + 1)
                src_ = pob[0:128, 0:QC] if k == 2 * NQC - 1 else pq0[:, :]
                scalar.copy(osb_[k % 2][:, :], src_).then_inc(sOS, 1)

            pending = -1  # iteration whose po_copy is pending
            for g in range(NG):
                i, t = g // NT, g % NT
                if asg[g] == 0:
                    na = preA[g + 1] - preA[i * NT]  # ACT-tile index within iter
                    scalar.wait_ge(sSC, g + 1)
                    if g >= 4:
                        scalar.wait_ge(sAV, g - 3)  # pt[g%4] free after av g-4
                    scalar.activation(
                        pt_[g % 4][:, :], s_[g % 3][:, :], EXP, 0.0, LN2
                    ).then_inc(sEXA, 1)
                    if i >= 1 and na == 4 and pending == i - 1:
                        po_copy(i - 1)
                        pending = -2
                if t == NT - 1:
                    if pending >= 0:
                        po_copy(pending)
                    pending = i
            po_copy(NI - 1)
            osb_copy_act(2 * NQC - 1)

        @block.vector
        def _(vector):
            # three of every four v copies, then k copies (f32r)
            for pb in range(NKB):
                if pb % 4 == 3:
                    continue
                vector.wait_ge(sPQ, pb + 1)
                pv = v_slots[pb % 8][0:128, 0:64]
                vector.tensor_copy(
                    v_all.ap()[:, pb, :, 0:Dh], pv[:, 0 : 2 * Dh]
                ).then_inc(sCP, 1)
            def k_copy(c):
                g = 2 * c
                sl = slice(c * QC, (c + 1) * QC)
                vector.wait_ge(sPQ, NKB + g + 1)
                vector.tensor_copy(
                    kT[:, sl], qk_slots[g % 4][0:128, 0:QC]
                ).then_inc(sCP, 1)

            for c_ in range(NQC):
                k_copy(c_)

            def exp_dve(g):
                vector.wait_ge(sSC, g + 1)
                if g >= 4:
                    vector.wait_ge(sAV, g - 3)
                vector.tensor_scalar(
                    pt_ap[g % 4][:, :].bitcast(i16),
                    s_[g % 3][:, :],
                    128.0,
                    16256.0 + SIGMA,
                    Alu.mult,
                    Alu.add,
                ).then_inc(sEXD, 1)

            def epi_recip(i):
                vector.wait_ge(sAV, NT * (i + 1))
                vector.reciprocal(rsb[:, :], po_den(i)).then_inc(sRC, 1)

            def epi_mul(i):
                h = i % 2
                c = i // 2
                qsl = slice(c * QC, (c + 1) * QC)
                vector.wait_ge(sPOC, i + 1)
                vector.tensor_mul(
                    oT_[h][:, qsl], po_sb_[i % 2][:, :], rsb[:, :]
                ).then_inc(sMU, 1)

            def epilogue(i):
                epi_recip(i)
                epi_mul(i)

            def osb_copy(k):
                vector.wait_ge(sPJ, k + 1)
                if k >= 2:
                    vector.wait_ge((sOD0, sOD1)[k % 2], 16 * (k // 2))
                src_ = pob[0:128, 0:QC] if k == 2 * NQC - 1 else pq0[:, :]
                vector.tensor_copy(osb_[k % 2][:, :], src_).then_inc(sOS, 1)

            for i in range(NI):
                for t in range(NT):
                    g = i * NT + t
                    if asg[g] == 1:
                        exp_dve(g)
                        nd = preD[g + 1] - preD[i * NT]
                        if i >= 1 and nd == 1:
                            epi_recip(i - 1)
                        if i >= 1 and nd == 2:
                            epi_mul(i - 1)
                        if i >= 3 and i % 2 == 1 and nd == 3:
                            osb_copy(i - 2)
                # osb copy for proj mc=0 lands right after this iteration's
                # last DVE exp so the PE's next sOS wait can resolve
                if i >= 2 and i % 2 == 0:
                    osb_copy(i - 2)
            epilogue(NI - 1)
            osb_copy(2 * NQC - 2)

    return nc


def _prep_in_maps(x, W_qkv, b_qkv, W_proj):
    import ml_dtypes

    bf = ml_dtypes.bfloat16
    in_maps = []
    for i in range(NCORES):
        b = i // 4
        heads = [2 * (i % 4), 2 * (i % 4) + 1]
        xT = np.ascontiguousarray(x[b].T.astype(np.float32))  # [256, 4096]

        def slc(base, scale):
            w = np.concatenate(
                [W_qkv[:, base + h * Dh : base + (h + 1) * Dh] for h in heads], axis=1
            ).astype(np.float32) * scale
            bb = np.concatenate(
                [b_qkv[base + h * Dh : base + (h + 1) * Dh] for h in heads]
            ).astype(np.float32)[None, :] * scale
            return w, bb

        wq, bq = slc(0, SCALE * LOG2E)
        wk, bk = slc(C, 1.0)
        wv, bv = slc(2 * C, 1.0)
        wp = np.concatenate(
            [W_proj[h * Dh : (h + 1) * Dh, :] for h in heads], axis=0
        ).astype(np.float32)  # [64, 256]

        def pad_qk(w64):
            # [rows, 64] -> [rows, 128]: head h dim d=2p+j -> col 64h+32j+p,
            # cols 64h+32j+16 .. +32 stay zero (DoubleRow pair padding)
            o = np.zeros((w64.shape[0], 128), dtype=np.float32)
            for h in (0, 1):
                wh = w64[:, 32 * h : 32 * h + 32]
                for j in (0, 1):
                    o[:, 64 * h + 32 * j : 64 * h + 32 * j + 16] = wh[:, j::2]
            return o

        wqp, wkp = pad_qk(wq), pad_qk(wk)
        bqp, bkp = pad_qk(bq), pad_qk(bk)

        wpack = np.zeros((128, WPK), dtype=np.float32)
        wpack[:, 0:128] = wqp[:128]
        wpack[:, 128:256] = wkp[:128]
        wpack[:, 256:320] = wv[:128]
        wpack[:, 320:448] = wqp[128:]
        wpack[:, 448:576] = wkp[128:]
        wpack[:, 576:640] = wv[128:]
        wpack[0, 640:768] = bqp[0]
        wpack[0, 768:896] = bkp[0]
        wpack[0, 896:960] = bv[0]
        wpack[0:Dh, 960:1216] = wp[:Dh]
        wpack[0:Dh, 1216:1472] = wp[Dh:]

        m = {
            "wpack": wpack.astype(bf),
            "x0": xT[:128].astype(bf),
            "x1": xT[128:].astype(bf),
        }
        in_maps.append({k: np.ascontiguousarray(v) for k, v in m.items()})
    return in_maps


LAST_RESULT = None


def kernel(x, W_qkv, b_qkv, W_proj, b_proj):
    global LAST_RESULT
    from concourse.bass_utils import run_bass_kernel_spmd

    if "nc" not in _CACHE:
        _CACHE["nc"] = _build_nc()
    nc = _CACHE["nc"]

    in_maps = _prep_in_maps(
        np.asarray(x), np.asarray(W_qkv), np.asarray(b_qkv), np.asarray(W_proj)
    )
    res = run_bass_kernel_spmd(nc, in_maps, core_ids=list(range(NCORES)))
    LAST_RESULT = res
    outs = res.results
    full = np.zeros((B, N, C), dtype=np.float32)
    for i in range(NCORES):
        b = i // 4
        full[b] += np.asarray(outs[i]["out"]).T
    full += np.asarray(b_proj).astype(np.float32)[None, None, :]
    return full
